# revision 17
# baseline (speedup 1.0000x reference)
"""Trainium2 Bass kernel for nn_EncoderSupernodes.

Self-contained: builds per-core inputs from the full (unsharded) inputs,
runs one SPMD Bass program on 8 NeuronCores, gathers the full output.

Sharding: core c handles sample c//2, supernode half c%2 (1024 supernodes).
kNN selection uses a group-max threshold (groups of 32 points): the
32nd-largest group-max of -d^2 upper-bounds the 32nd-nearest distance; all
points passing the threshold are mean-pooled with the exact count
(validated end-to-end rel err ~7e-5 vs exact top-32 reference).
"""
import numpy as np

import concourse.bass as bass
import concourse.mybir as mybir
import concourse.tile as tile
from concourse import bacc
from concourse.bass_utils import run_bass_kernel_spmd
from concourse.masks import make_identity

F32 = mybir.dt.float32
AF = mybir.ActivationFunctionType
ALU = mybir.AluOpType

B, N, S, K = 4, 32768, 2048, 32
NDIM, GNN, ENC, HEADS, DEPTH = 3, 256, 384, 6, 4
LN_EPS = 1e-6
SOWN = S // 2
HALF = 42
NCH = N // 128
NCH512 = N // 512
TOK = SOWN
HD = ENC // HEADS

N_ANG = 123   # device ang rows: [0,117): (c, i>=3); [117,123): (c, i in {1,2})
N_SIN = 117   # sin computed on device for rows [0,117)


def _freqs():
    return np.exp(np.arange(HALF) * (-np.log(10000.0) / (HALF - 1))).astype(np.float64)


def _ang_rows():
    rows = []
    for c in range(3):
        for i in range(3, HALF):
            rows.append((c, i))
    for c in range(3):
        for i in (1, 2):
            rows.append((c, i))
    return rows


def build_nc(dbg=False, pmax=99, nocc=False):
    nc = bacc.Bacc()

    def din(name, shape):
        return nc.dram_tensor(name, list(shape), F32, kind="ExternalInput")

    x5_d = din("x5", (5, N))
    sna1_d = din("sna1", (128, SOWN))
    sna5_d = din("sna5", (128, SOWN))
    ehi_d = din("ehi", (128, 11264))
    smat_d = din("smat", (128, N_ANG))
    pw1_d = din("pw1", (128, GNN))
    w1s_d = din("w1s", (N_SIN, GNN))
    w1c_d = din("w1c", (N_ANG, GNN))
    w1hi_d = din("w1hi", (128, GNN))
    b1_d = din("b1", (128, 2))
    w2_d = din("w2", (128, 2, GNN))
    projw_d = din("projw", (128, 2, ENC))
    projb_d = din("projb", (128, 3))
    blk_d = []
    for k in range(DEPTH):
        blk_d.append({
            "wq": din(f"wq{k}", (128, 3, ENC)), "bq": din(f"bq{k}", (128, 3)),
            "wk": din(f"wk{k}", (128, 3, ENC)), "bk": din(f"bk{k}", (128, 3)),
            "wvp": din(f"wvp{k}", (128, 3, 390)),
            "woh": din(f"woh{k}", (64, HEADS * ENC)),
            "bo": din(f"bo{k}", (128, 3)),
            "ln1s": din(f"ln1s{k}", (128, 3)), "ln1b": din(f"ln1b{k}", (128, 3)),
            "ln2s": din(f"ln2s{k}", (128, 3)), "ln2b": din(f"ln2b{k}", (128, 3)),
            "w1m": din(f"w1m{k}", (128, 3, 4 * ENC)), "b1m": din(f"b1m{k}", (128, 12)),
            "w2m": din(f"w2m{k}", (128, 12, ENC)), "b2m": din(f"b2m{k}", (128, 3)),
        })

    out_d = nc.dram_tensor("out_x", [TOK, ENC], F32, kind="ExternalOutput")
    dbg_d = {}
    if dbg:
        dbg_d["tau"] = nc.dram_tensor("dbg_tau", [128, 8], F32, kind="ExternalOutput")
        dbg_d["cnt"] = nc.dram_tensor("dbg_cnt", [128, 8], F32, kind="ExternalOutput")
        dbg_d["x0"] = nc.dram_tensor("dbg_x0", [128, 3, 1024], F32, kind="ExternalOutput")

    f_dram = nc.dram_tensor("f_scratch", [N, GNN], F32)
    cc_in = [nc.dram_tensor(f"cc_in{k}", [ENC, TOK], F32) for k in range(DEPTH)]
    cc_out = [nc.dram_tensor(f"cc_out{k}", [2, ENC, TOK], F32) for k in range(DEPTH)]
    groups = [[0, 1], [2, 3], [4, 5], [6, 7]]

    with tile.TileContext(nc) as tc:
        with tc.tile_pool(name="const", bufs=1) as cp:
            ident = cp.tile([128, 128], F32, tag="ident")
            make_identity(nc, ident)
            ones_col = cp.tile([128, 1], F32, tag="onec")
            nc.vector.memset(ones_col[:], 1.0)
            ones_row = cp.tile([1, 128], F32, tag="oner")
            nc.vector.memset(ones_row[:], 1.0)
            ones_at64 = cp.tile([128, 64], F32, tag="one64")
            nc.vector.memset(ones_at64[:], 1.0)
            halfpi = cp.tile([128, 1], F32, tag="hpi")
            nc.vector.memset(halfpi[:], float(np.pi / 2))
            zcol = cp.tile([128, 1], F32, tag="zc")
            nc.vector.memset(zcol[:], 0.0)
            x_sb = cp.tile([128, 3, TOK], F32, tag="xa")

            # ======================= GNN phases (1-4) =======================
            with tc.tile_pool(name="gnn", bufs=1) as gp:
                x5 = gp.tile([128, 12288], F32, tag="x5")
                for g in range(8):
                    nc.sync.dma_start(
                        x5[32 * (g % 3):32 * (g % 3) + 5,
                           (g // 3) * 4096:(g // 3) * 4096 + 4096],
                        x5_d[:, g * 4096:(g + 1) * 4096])
                sna1 = gp.tile([128, SOWN], F32, tag="sna1")
                nc.sync.dma_start(sna1[:], sna1_d[:])
                sna5 = gp.tile([128, SOWN], F32, tag="sna5")
                nc.sync.dma_start(sna5[:], sna5_d[:])
                ehi = gp.tile([128, 11264], F32, tag="ehi")
                nc.sync.dma_start(ehi[:], ehi_d[:])
                smat = gp.tile([128, N_ANG], F32, tag="smat")
                nc.sync.dma_start(smat[:], smat_d[:])
                pw1 = gp.tile([128, GNN], F32, tag="pw1")
                nc.sync.dma_start(pw1[:], pw1_d[:])
                w1s = gp.tile([N_SIN, GNN], F32, tag="w1s")
                nc.sync.dma_start(w1s[:], w1s_d[:])
                w1c = gp.tile([N_ANG, GNN], F32, tag="w1c")
                nc.sync.dma_start(w1c[:], w1c_d[:])
                w1hi = gp.tile([128, GNN], F32, tag="w1hi")
                nc.sync.dma_start(w1hi[:], w1hi_d[:])
                b1 = gp.tile([128, 2], F32, tag="b1")
                nc.sync.dma_start(b1[:], b1_d[:])
                w2 = gp.tile([128, 2, GNN], F32, tag="w2")
                nc.sync.dma_start(w2[:], w2_d[:])
                projw = gp.tile([128, 2, ENC], F32, tag="projw")
                nc.sync.dma_start(projw[:], projw_d[:])
                projb = gp.tile([128, 3], F32, tag="projb")
                nc.sync.dma_start(projb[:], projb_d[:])

                tau_use = gp.tile([128, 8], F32, tag="tau")
                h1acc = gp.tile([128, 2, NCH512], F32, tag="h1acc")
                sfh = gp.tile([128, 2], F32, tag="sfh")
                pooled01 = gp.tile([128, 2, SOWN], F32, tag="pooled")

                # ---- phase 1: distances, group-max, threshold ----
                with tc.tile_pool(name="p1", bufs=2) as p1, \
                     tc.tile_pool(name="p1ps", bufs=2, space="PSUM") as p1ps:
                    for sb in range(8):
                        M = p1.tile([128, 1024], F32, tag="M")
                        for ch in range(NCH512):
                            g = ch // 8
                            b = g % 3
                            off = (g // 3) * 4096 + (ch % 8) * 512
                            g_ps = p1ps.tile([128, 512], F32, tag="g")
                            nc.tensor.matmul(
                                g_ps[:],
                                sna1[32 * b:32 * b + 5, sb * 128:(sb + 1) * 128],
                                x5[32 * b:32 * b + 5, off:off + 512],
                                start=True, stop=True)
                            nc.vector.tensor_reduce(
                                out=M[:, ch * 16:(ch + 1) * 16],
                                in_=g_ps.rearrange("p (g e) -> p g e", e=32),
                                axis=mybir.AxisListType.X,
                                op=ALU.max)
                        t8 = p1.tile([128, 8], F32, tag="t8")
                        for r in range(4):
                            nc.vector.max(out=t8[:], in_=M[:])
                            if r < 3:
                                nc.vector.match_replace(out=M[:], in_to_replace=t8[:],
                                                        in_values=M[:], imm_value=-1e30)
                        nc.vector.tensor_copy(tau_use[:, sb:sb + 1], t8[:, 7:8])
                    tabs = p1.tile([128, 8], F32, tag="tabs")
                    nc.scalar.activation(tabs[:], tau_use[:], AF.Abs, bias=zcol[:])
                    nc.vector.tensor_scalar(tabs[:], tabs[:], 1e-5, 1e-5, op0=ALU.mult, op1=ALU.add)
                    nc.vector.tensor_sub(tau_use[:], tau_use[:], tabs[:])
                    if dbg:
                        nc.sync.dma_start(dbg_d["tau"][:], tau_use[:])
                    taur_ps = p1ps.tile([1, 1024], F32, tag="taur")
                    for sb in range(8):
                        tcv = p1.tile([128, 1], F32, tag="tc")
                        nc.vector.tensor_copy(tcv[:], tau_use[:, sb:sb + 1])
                        nc.tensor.transpose(taur_ps[0:1, sb * 128:(sb + 1) * 128], tcv[:], ident[:])
                    taur = p1.tile([1, 1024], F32, tag="taurs")
                    nc.scalar.mul(taur[:], taur_ps[:], -1.0)
                    for b in range(3):
                        nc.sync.dma_start(sna5[32 * b + 4:32 * b + 5, :], taur[:])

                # ---- phase 2: featurize all points -> F in DRAM ----
                if pmax >= 2:
                 with tc.tile_pool(name="p2", bufs=2) as p2, \
                     tc.tile_pool(name="p2e", bufs=9) as p2e, \
                     tc.tile_pool(name="p2ps", bufs=2, space="PSUM") as p2ps, \
                     tc.tile_pool(name="p2f", bufs=3) as p2f:
                    for sup in range(8):
                        embs = []
                        for c512 in range(8):
                            ch = sup * 8 + c512
                            g = ch // 8
                            b = g % 3
                            coloff = (g // 3) * 4096 + (ch % 8) * 512
                            ang_ps = p2ps.tile([N_ANG, 512], F32, tag="ang", bufs=1)
                            nc.tensor.matmul(
                                ang_ps[:], smat[32 * b:32 * b + 5, :],
                                x5[32 * b:32 * b + 5, coloff:coloff + 512],
                                start=True, stop=True)
                            ang = p2.tile([N_ANG, 512], F32, tag="angs")
                            nc.scalar.copy(ang[:], ang_ps[:])
                            sint = p2e.tile([N_SIN, 512], F32, tag="sin")
                            nc.scalar.activation(sint[:], ang[0:N_SIN, :], AF.Sin,
                                                 bias=zcol[0:N_SIN, :])
                            aabs = p2.tile([N_ANG, 512], F32, tag="aabs")
                            nc.scalar.activation(aabs[:], ang[:], AF.Abs, bias=zcol[0:N_ANG, :])
                            cost = p2e.tile([N_ANG, 512], F32, tag="cos")
                            nc.scalar.activation(cost[:], aabs[:], AF.Sin,
                                                 bias=halfpi[0:N_ANG, :], scale=-1.0)
                            embs.append((sint, cost))
                        for c512 in range(8):
                            ch = sup * 8 + c512
                            g = ch // 8
                            b = g % 3
                            coloff = (g // 3) * 4096 + (ch % 8) * 512
                            m = ch % 3
                            ecol = (ch // 3) * 512
                            sint, cost = embs[c512]
                            l1_ps = [p2ps.tile([128, 512], F32, tag=f"l1_{p}", name=f"l1ps{p}") for p in range(2)]
                            for p in range(2):
                                pc = slice(p * 128, (p + 1) * 128)
                                nc.tensor.matmul(l1_ps[p][:], pw1[32 * b:32 * b + 5, pc],
                                                 x5[32 * b:32 * b + 5, coloff:coloff + 512],
                                                 start=True, stop=False)
                                nc.tensor.matmul(l1_ps[p][:], w1s[:, pc], sint[:],
                                                 start=False, stop=False)
                                nc.tensor.matmul(l1_ps[p][:], w1c[:, pc], cost[:],
                                                 start=False, stop=False)
                                nc.tensor.matmul(l1_ps[p][:], w1hi[32 * m:32 * m + 12, pc],
                                                 ehi[32 * m:32 * m + 12, ecol:ecol + 512],
                                                 start=False, stop=True)
                            h1 = [p2.tile([128, 512], F32, tag=f"h1_{p}", name=f"h1t{p}") for p in range(2)]
                            for p in range(2):
                                nc.scalar.activation(h1[p][:], l1_ps[p][:], AF.Gelu_apprx_tanh,
                                                     bias=b1[:, p:p + 1],
                                                     accum_out=h1acc[:, p, ch:ch + 1])
                            for sub in range(4):
                                l2_ps = p2ps.tile([128, GNN], F32, tag="l2")
                                for ccf in range(2):
                                    nc.tensor.matmul(l2_ps[:], h1[ccf][:, sub * 128:(sub + 1) * 128],
                                                     w2[:, ccf, :], start=(ccf == 0), stop=(ccf == 1))
                                fch = p2f.tile([128, GNN], F32, tag="f")
                                nc.scalar.copy(fch[:], l2_ps[:])
                                nc.sync.dma_start(
                                    f_dram[(ch * 4 + sub) * 128:(ch * 4 + sub + 1) * 128, :], fch[:])
                    h1tot = p2.tile([128, 2], F32, tag="h1tot")
                    for p in range(2):
                        nc.vector.tensor_reduce(out=h1tot[:, p:p + 1], in_=h1acc[:, p, :],
                                                axis=mybir.AxisListType.X, op=ALU.add)
                    sf_ps = p2ps.tile([128, 2], F32, tag="sf", bufs=1)
                    for p in range(2):
                        for ccf in range(2):
                            nc.tensor.matmul(sf_ps[:, p:p + 1], w2[:, ccf, p * 128:(p + 1) * 128],
                                             h1tot[:, ccf:ccf + 1], start=(ccf == 0), stop=(ccf == 1))
                    nc.vector.tensor_scalar_mul(sfh[:], sf_ps[:], 0.5)

                # ---- phase 3: mask + pooled sums + count ----
                if pmax >= 3:
                 with tc.tile_pool(name="p3", bufs=2) as p3:
                  with tc.tile_pool(name="p3acc", bufs=1, space="PSUM") as p3acc, \
                       tc.tile_pool(name="p3f", bufs=3) as p3f:
                    p3ps = p3acc
                    pool_ps = [p3acc.tile([128, SOWN], F32, tag=f"pool{p}", name=f"poolps{p}") for p in range(2)]
                    cnt_ps = p3acc.tile([1, SOWN], F32, tag="cnt")
                    for g4 in range(NCH // 4):
                        f4 = p3f.tile([128, 4, GNN], F32, tag="f4")
                        nc.sync.dma_start(
                            f4[:],
                            f_dram[g4 * 512:(g4 + 1) * 512, :].rearrange("(q p) f -> p q f", p=128))
                        for q in range(4):
                            j = g4 * 4 + q
                            gg = j // 32
                            b = gg % 3
                            noff = (gg // 3) * 4096 + (j % 32) * 128
                            mask_ps = p3acc.tile([128, SOWN], F32, tag="mask")
                            for hh in range(2):
                                nc.tensor.matmul(
                                    mask_ps[:, hh * 512:(hh + 1) * 512],
                                    x5[32 * b:32 * b + 5, noff:noff + 128],
                                    sna5[32 * b:32 * b + 5, hh * 512:(hh + 1) * 512],
                                    start=True, stop=True)
                            mask = p3.tile([128, SOWN], F32, tag="mask_sb")
                            nc.scalar.activation(mask[:], mask_ps[:], AF.Sign, bias=zcol[:])
                            first, last = (j == 0), (j == NCH - 1)
                            for p in range(2):
                                for hh in range(2):
                                    nc.tensor.matmul(
                                        pool_ps[p][:, hh * 512:(hh + 1) * 512],
                                        f4[:, q, p * 128:(p + 1) * 128],
                                        mask[:, hh * 512:(hh + 1) * 512],
                                        start=first, stop=last)
                            for hh in range(2):
                                nc.tensor.matmul(
                                    cnt_ps[:, hh * 512:(hh + 1) * 512],
                                    ones_col[:],
                                    mask[:, hh * 512:(hh + 1) * 512],
                                    start=first, stop=last)
                    for p in range(2):
                        nc.vector.tensor_scalar(pooled01[:, p, :], pool_ps[p][:],
                                                0.5, sfh[:, p:p + 1], op0=ALU.mult, op1=ALU.add)
                    cntr = p3.tile([1, SOWN], F32, tag="cntr")
                    nc.scalar.copy(cntr[:], cnt_ps[:])
                  with tc.tile_pool(name="p3ps2", bufs=1, space="PSUM") as p3ps2:
                    cntc_ps = p3ps2.tile([128, 8], F32, tag="cntc")
                    for sb in range(8):
                        nc.tensor.transpose(cntc_ps[:, sb:sb + 1],
                                            cntr[0:1, sb * 128:(sb + 1) * 128], ident[0:1, 0:1])
                    cntc = p3.tile([128, 8], F32, tag="cntc_sb")
                    nc.vector.tensor_scalar(cntc[:], cntc_ps[:], 0.5, float(N / 2),
                                            op0=ALU.mult, op1=ALU.add)
                    if dbg:
                        nc.sync.dma_start(dbg_d["cnt"][:], cntc[:])
                    rcpc = p3.tile([128, 8], F32, tag="rcpc")
                    nc.vector.reciprocal(rcpc[:], cntc[:])
                    rcpr_ps = p3ps2.tile([1, SOWN], F32, tag="rcpr")
                    for sb in range(8):
                        rcol = p3.tile([128, 1], F32, tag="rcol")
                        nc.vector.tensor_copy(rcol[:], rcpc[:, sb:sb + 1])
                        nc.tensor.transpose(rcpr_ps[0:1, sb * 128:(sb + 1) * 128], rcol[:], ident[:])
                    rcpr = p3.tile([1, SOWN], F32, tag="rcprs")
                    nc.scalar.copy(rcpr[:], rcpr_ps[:])
                    rbc_ps = p3ps2.tile([128, SOWN], F32, tag="rbc")
                    for hh in range(2):
                        nc.tensor.matmul(rbc_ps[:, hh * 512:(hh + 1) * 512], ones_row[:],
                                         rcpr[:, hh * 512:(hh + 1) * 512], start=True, stop=True)
                    rbc = p3.tile([128, SOWN], F32, tag="rbcs")
                    nc.scalar.copy(rbc[:], rbc_ps[:])
                    for pb in range(3):
                        pr_ps = p3ps2.tile([128, SOWN], F32, tag="prj")
                        for ccf in range(2):
                            for hh in range(2):
                                nc.tensor.matmul(pr_ps[:, hh * 512:(hh + 1) * 512],
                                                 projw[:, ccf, pb * 128:(pb + 1) * 128],
                                                 pooled01[:, ccf, hh * 512:(hh + 1) * 512],
                                                 start=(ccf == 0), stop=(ccf == 1))
                        nc.vector.tensor_mul(x_sb[:, pb, :], pr_ps[:], rbc[:])
                        nc.vector.tensor_scalar_add(x_sb[:, pb, :], x_sb[:, pb, :], projb[:, pb:pb + 1])
                    if dbg:
                        nc.sync.dma_start(dbg_d["x0"][:], x_sb[:])

            # ======================= transformer blocks =======================
            if pmax < 4:
                nc.vector.memset(x_sb[:], 0.0)
            for kblk in range(min(max(pmax - 3, 0), DEPTH)):
                bd = blk_d[kblk]
                with tc.tile_pool(name=f"bw{kblk}", bufs=1) as bw, \
                     tc.tile_pool(name=f"bw2{kblk}", bufs=1) as bw2, \
                     tc.tile_pool(name=f"bt{kblk}", bufs=1) as bt, \
                     tc.tile_pool(name=f"bs{kblk}", bufs=2) as bs:
                    wq = bw.tile([128, 3, ENC], F32, tag="wq")
                    nc.sync.dma_start(wq[:], bd["wq"][:])
                    bq = bw.tile([128, 3], F32, tag="bq")
                    nc.sync.dma_start(bq[:], bd["bq"][:])
                    wk = bw.tile([128, 3, ENC], F32, tag="wk")
                    nc.sync.dma_start(wk[:], bd["wk"][:])
                    bk = bw.tile([128, 3], F32, tag="bk")
                    nc.sync.dma_start(bk[:], bd["bk"][:])
                    wvp = bw.tile([128, 3, 390], F32, tag="wvp")
                    nc.sync.dma_start(wvp[:], bd["wvp"][:])
                    woh = bw.tile([64, HEADS * ENC], F32, tag="woh")
                    nc.sync.dma_start(woh[:], bd["woh"][:])
                    bo = bw.tile([128, 3], F32, tag="bo")
                    nc.sync.dma_start(bo[:], bd["bo"][:])
                    ln1s = bw.tile([128, 3], F32, tag="ln1s")
                    nc.sync.dma_start(ln1s[:], bd["ln1s"][:])
                    ln1b = bw.tile([128, 3], F32, tag="ln1b")
                    nc.sync.dma_start(ln1b[:], bd["ln1b"][:])
                    ln2s = bw.tile([128, 3], F32, tag="ln2s")
                    nc.sync.dma_start(ln2s[:], bd["ln2s"][:])
                    ln2b = bw.tile([128, 3], F32, tag="ln2b")
                    nc.sync.dma_start(ln2b[:], bd["ln2b"][:])
                    b1m = bw.tile([128, 12], F32, tag="b1m")
                    nc.sync.dma_start(b1m[:], bd["b1m"][:])
                    b2m = bw.tile([128, 3], F32, tag="b2m")
                    nc.sync.dma_start(b2m[:], bd["b2m"][:])

                    def layernorm(x_in, s_col, b_col, out_tag):
                        T = x_in.shape[2]
                        tbs = T // 128
                        with tc.tile_pool(name="lnps", bufs=1, space="PSUM") as lps:
                            mu_ps = lps.tile([128, tbs], F32, tag="mu")
                            m2_ps = lps.tile([128, tbs], F32, tag="m2")
                            for tb in range(tbs):
                                sq = bs.tile([128, 3, 128], F32, tag="sq", bufs=1)
                                for pb in range(3):
                                    nc.vector.tensor_mul(sq[:, pb, :],
                                                         x_in[:, pb, tb * 128:(tb + 1) * 128],
                                                         x_in[:, pb, tb * 128:(tb + 1) * 128])
                                for pb in range(3):
                                    nc.tensor.matmul(mu_ps[:, tb:tb + 1],
                                                     x_in[:, pb, tb * 128:(tb + 1) * 128],
                                                     ones_col[:], start=(pb == 0), stop=(pb == 2))
                                for pb in range(3):
                                    nc.tensor.matmul(m2_ps[:, tb:tb + 1], sq[:, pb, :],
                                                     ones_col[:], start=(pb == 0), stop=(pb == 2))
                            mean = bs.tile([128, 8], F32, tag="mean")
                            nc.vector.tensor_scalar_mul(mean[:, 0:tbs], mu_ps[:], 1.0 / ENC)
                            var = bs.tile([128, 8], F32, tag="var")
                            nc.vector.tensor_scalar_mul(var[:, 0:tbs], m2_ps[:], 1.0 / ENC)
                            msq = bs.tile([128, 8], F32, tag="msq")
                            nc.vector.tensor_mul(msq[:, 0:tbs], mean[:, 0:tbs], mean[:, 0:tbs])
                            nc.vector.tensor_sub(var[:, 0:tbs], var[:, 0:tbs], msq[:, 0:tbs])
                            nc.vector.tensor_scalar_add(var[:, 0:tbs], var[:, 0:tbs], LN_EPS)
                            rstd = bs.tile([128, 8], F32, tag="rstd")
                            nc.scalar.activation(rstd[:, 0:tbs], var[:, 0:tbs],
                                                 AF.Abs_reciprocal_sqrt, bias=zcol[:])
                            rr = bs.tile([128, 8], F32, tag="rr")
                            nc.vector.tensor_mul(rr[:, 0:tbs], rstd[:, 0:tbs], rstd[:, 0:tbs])
                            nc.vector.tensor_mul(rr[:, 0:tbs], rr[:, 0:tbs], var[:, 0:tbs])
                            nc.vector.tensor_scalar(rr[:, 0:tbs], rr[:, 0:tbs], -0.5, 1.5,
                                                    op0=ALU.mult, op1=ALU.add)
                            nc.vector.tensor_mul(rstd[:, 0:tbs], rstd[:, 0:tbs], rr[:, 0:tbs])
                            negmr = bs.tile([128, 8], F32, tag="negmr")
                            nc.vector.tensor_mul(negmr[:, 0:tbs], mean[:, 0:tbs], rstd[:, 0:tbs])
                            nc.vector.tensor_scalar_mul(negmr[:, 0:tbs], negmr[:, 0:tbs], -1.0)
                            hout = bt.tile([128, 3, T], F32, tag=out_tag)
                            for phase in range(2):
                                coef = rstd if phase == 0 else negmr
                                row_ps = lps.tile([1, T], F32, tag="lrow", bufs=1)
                                for tb in range(tbs):
                                    cv = bs.tile([128, 1], F32, tag="cv", bufs=2)
                                    nc.vector.tensor_copy(cv[:], coef[:, tb:tb + 1])
                                    nc.tensor.transpose(row_ps[0:1, tb * 128:(tb + 1) * 128],
                                                        cv[:], ident[:])
                                rowsb = bs.tile([1, T], F32, tag="lrows", bufs=1)
                                nc.scalar.copy(rowsb[:], row_ps[:])
                                bc_ps = lps.tile([128, T], F32, tag="lbc", bufs=1)
                                for hh in range(T // 512):
                                    nc.tensor.matmul(bc_ps[:, hh * 512:(hh + 1) * 512], ones_row[:],
                                                     rowsb[:, hh * 512:(hh + 1) * 512],
                                                     start=True, stop=True)
                                for pb in range(3):
                                    if phase == 0:
                                        nc.vector.tensor_mul(hout[:, pb, :], x_in[:, pb, :], bc_ps[:])
                                    else:
                                        nc.vector.tensor_add(hout[:, pb, :], hout[:, pb, :], bc_ps[:])
                            for pb in range(3):
                                nc.scalar.activation(hout[:, pb, :], hout[:, pb, :], AF.Identity,
                                                     bias=b_col[:, pb:pb + 1], scale=s_col[:, pb:pb + 1])
                        return hout

                    h_own = layernorm(x_sb, ln1s, ln1b, "t12")
                    nc.gpsimd.dma_start(
                        cc_in[kblk].rearrange("(pb p) t -> p pb t", p=128)[:], h_own[:])
                    if nocc:
                        nc.gpsimd.dma_start(cc_out[kblk][0], cc_in[kblk][:])
                        nc.gpsimd.dma_start(cc_out[kblk][1], cc_in[kblk][:])
                    else:
                        nc.gpsimd.collective_compute(
                            "AllGather", ALU.bypass, replica_groups=groups,
                            ins=[cc_in[kblk][:]], outs=[cc_out[kblk][:]])
                    q_t = bt.tile([128, 3, TOK], F32, tag="q_t")
                    h_full = bt.tile([128, 3, S], F32, tag="hfull")
                    k_t = bt.tile([128, 3, S], F32, tag="k_t")
                    vt = bt.tile([128, 16, 390], F32, tag="vt")
                    with tc.tile_pool(name="qkvps", bufs=2, space="PSUM") as qps:
                        for pb in range(3):
                            qp = qps.tile([128, TOK], F32, tag="qp")
                            for ccf in range(3):
                                for hh in range(TOK // 512):
                                    nc.tensor.matmul(qp[:, hh * 512:(hh + 1) * 512],
                                                     wq[:, ccf, pb * 128:(pb + 1) * 128],
                                                     h_own[:, ccf, hh * 512:(hh + 1) * 512],
                                                     start=(ccf == 0), stop=(ccf == 2))
                            nc.scalar.activation(q_t[:, pb, :], qp[:], AF.Identity,
                                                 bias=bq[:, pb:pb + 1], scale=1.0)
                        for chf in range(2):
                            for pb in range(3):
                                nc.sync.dma_start(h_full[:, pb, chf * TOK:(chf + 1) * TOK],
                                                  cc_out[kblk][chf, pb * 128:(pb + 1) * 128, :])
                        for pb in range(3):
                            for half in range(2):
                                kp = qps.tile([128, TOK], F32, tag="kp", bufs=1)
                                for ccf in range(3):
                                    for hh in range(TOK // 512):
                                        nc.tensor.matmul(
                                            kp[:, hh * 512:(hh + 1) * 512],
                                            wk[:, ccf, pb * 128:(pb + 1) * 128],
                                            h_full[:, ccf, half * TOK + hh * 512:half * TOK + (hh + 1) * 512],
                                            start=(ccf == 0), stop=(ccf == 2))
                                nc.scalar.activation(k_t[:, pb, half * TOK:(half + 1) * TOK], kp[:],
                                                     AF.Identity, bias=bk[:, pb:pb + 1], scale=1.0)
                        for tb in range(16):
                            vp = qps.tile([128, 390], F32, tag="vp")
                            for ccf in range(3):
                                nc.tensor.matmul(vp[:],
                                                 h_full[:, ccf, tb * 128:(tb + 1) * 128],
                                                 wvp[:, ccf, :], start=(ccf == 0), stop=(ccf == 2))
                            nc.scalar.copy(vt[:, tb, :], vp[:])
                            ones_slots = vt[:, tb, :].rearrange("p (h e) -> p h e", e=65)[:, :, 64:65]
                            nc.vector.memset(ones_slots, 1.0)
                    o_all = bt.tile([64, HEADS, TOK], F32, tag="hfull")
                    with tc.tile_pool(name="attps", bufs=2, space="PSUM") as aps:
                        for h in range(HEADS):
                            pbh = (64 * h) // 128
                            offh = (64 * h) % 128
                            av_ps = aps.tile([65, TOK], F32, tag="av", bufs=1)
                            for kb in range(16):
                                sc_ps = aps.tile([128, TOK], F32, tag="sc")
                                for hh in range(TOK // 512):
                                    nc.tensor.matmul(sc_ps[:, hh * 512:(hh + 1) * 512],
                                                     k_t[offh:offh + 64, pbh, kb * 128:(kb + 1) * 128],
                                                     q_t[offh:offh + 64, pbh, hh * 512:(hh + 1) * 512],
                                                     start=True, stop=True)
                                esb = bs.tile([128, TOK], F32, tag="esb")
                                nc.scalar.activation(esb[:], sc_ps[:], AF.Exp, bias=zcol[:])
                                for hh in range(TOK // 512):
                                    nc.tensor.matmul(av_ps[:, hh * 512:(hh + 1) * 512],
                                                     vt[:, kb, 65 * h:65 * h + 65],
                                                     esb[:, hh * 512:(hh + 1) * 512],
                                                     start=(kb == 0), stop=(kb == 15))
                            rz = bs.tile([65, TOK], F32, tag="zr", bufs=1)
                            nc.scalar.activation(rz[64:65, :], av_ps[64:65, :], AF.Ln,
                                                 bias=zcol[64:65, :])
                            nc.scalar.activation(rz[64:65, :], rz[64:65, :], AF.Exp,
                                                 bias=zcol[64:65, :], scale=-1.0)
                            oh = bs.tile([64, TOK], F32, tag="oh")
                            nc.scalar.copy(oh[:], av_ps[0:64, :])
                            rzb_ps = aps.tile([64, TOK], F32, tag="rzb", bufs=1)
                            for hh in range(TOK // 512):
                                nc.tensor.matmul(rzb_ps[:, hh * 512:(hh + 1) * 512],
                                                 ones_at64[64:65, :],
                                                 rz[64:65, hh * 512:(hh + 1) * 512],
                                                 start=True, stop=True)
                            nc.vector.tensor_mul(o_all[:, h, :], oh[:], rzb_ps[:])
                    x2 = bt.tile([128, 3, TOK], F32, tag="q_t")
                    with tc.tile_pool(name="ops", bufs=2, space="PSUM") as ops_:
                        for pb in range(3):
                            att_ps = ops_.tile([128, TOK], F32, tag="att")
                            for h in range(HEADS):
                                for hh in range(TOK // 512):
                                    nc.tensor.matmul(att_ps[:, hh * 512:(hh + 1) * 512],
                                                     woh[:, h * ENC + pb * 128:h * ENC + (pb + 1) * 128],
                                                     o_all[:, h, hh * 512:(hh + 1) * 512],
                                                     start=(h == 0), stop=(h == HEADS - 1))
                            ao = bs.tile([128, TOK], F32, tag="evt")
                            nc.scalar.activation(ao[:], att_ps[:], AF.Identity,
                                                 bias=bo[:, pb:pb + 1], scale=1.0)
                            nc.vector.tensor_add(x2[:, pb, :], x_sb[:, pb, :], ao[:])
                    h2 = layernorm(x2, ln2s, ln2b, "t12")
                    with tc.tile_pool(name="mlpps", bufs=1, space="PSUM") as mps_:
                        out2_ps = [mps_.tile([128, TOK], F32, tag=f"o2{pb}", name=f"o2ps{pb}")
                                   for pb in range(3)]
                        for halfm in range(2):
                            w1h = bw2.tile([128, 3, 768], F32, tag="w1h")
                            nc.sync.dma_start(w1h[:], bd["w1m"][:, :, halfm * 768:(halfm + 1) * 768])
                            w2h = bw2.tile([128, 6, ENC], F32, tag="w2h")
                            nc.sync.dma_start(w2h[:], bd["w2m"][:, halfm * 6:(halfm + 1) * 6, :])
                            mh = bt.tile([128, 6, TOK], F32, tag="hfull")
                            for pbm in range(6):
                                pg = halfm * 6 + pbm
                                m_ps = mps_.tile([128, TOK], F32, tag="mps")
                                for ccf in range(3):
                                    for hh in range(TOK // 512):
                                        nc.tensor.matmul(m_ps[:, hh * 512:(hh + 1) * 512],
                                                         w1h[:, ccf, pbm * 128:(pbm + 1) * 128],
                                                         h2[:, ccf, hh * 512:(hh + 1) * 512],
                                                         start=(ccf == 0), stop=(ccf == 2))
                                nc.scalar.activation(mh[:, pbm, :], m_ps[:], AF.Gelu_apprx_tanh,
                                                     bias=b1m[:, pg:pg + 1])
                            for pb in range(3):
                                for hc in range(6):
                                    hg = halfm * 6 + hc
                                    for hh in range(TOK // 512):
                                        nc.tensor.matmul(out2_ps[pb][:, hh * 512:(hh + 1) * 512],
                                                         w2h[:, hc, pb * 128:(pb + 1) * 128],
                                                         mh[:, hc, hh * 512:(hh + 1) * 512],
                                                         start=(hg == 0), stop=(hg == 11))
                        xnew = cp.tile([128, 3, TOK], F32, tag=("xb" if kblk % 2 == 0 else "xa"),
                                       name=f"xnew{kblk}")
                        for pb in range(3):
                            mo = bs.tile([128, TOK], F32, tag="evt")
                            nc.scalar.activation(mo[:], out2_ps[pb][:], AF.Identity,
                                                 bias=b2m[:, pb:pb + 1], scale=1.0)
                            nc.vector.tensor_add(xnew[:, pb, :], x2[:, pb, :], mo[:])
                        x_sb = xnew

            # ======================= output transpose + store =======================
            with tc.tile_pool(name="p6", bufs=2) as p6, \
                 tc.tile_pool(name="p6ps", bufs=2, space="PSUM") as p6ps:
                out_tok = p6.tile([128, 8, ENC], F32, tag="ot")
                for tb in range(8):
                    for fc in range(3):
                        tp = p6ps.tile([128, 128], F32, tag="tp")
                        nc.tensor.transpose(tp[:], x_sb[:, fc, tb * 128:(tb + 1) * 128], ident[:])
                        nc.scalar.copy(out_tok[:, tb, fc * 128:(fc + 1) * 128], tp[:])
                nc.sync.dma_start(out_d.rearrange("(tb p) f -> p tb f", p=128)[:], out_tok[:])

    nc.compile()
    return nc


# ====================== host-side preparation ======================

_NC_CACHE = {}


def _get_nc(dbg=False, pmax=99, nocc=False):
    key = (dbg, pmax, nocc)
    if key not in _NC_CACHE:
        _NC_CACHE[key] = build_nc(dbg, pmax, nocc)
    return _NC_CACHE[key]


def _prep_weights(params):
    g = lambda k: np.asarray(params[k], np.float32)
    freqs = _freqs()
    rows = _ang_rows()
    msg_w1 = g("msg_w1")
    d = {}
    smat = np.zeros((128, N_ANG), np.float32)
    for k, (c, i) in enumerate(rows):
        for b in range(3):
            smat[32 * b + c, k] = freqs[i]
    d["smat"] = smat
    pm = np.zeros((5, GNN), np.float32)
    pm[0:3] = g("pool_w")
    pm[4] = g("pool_b")
    pw1 = (pm @ msg_w1).astype(np.float32)
    pw1r = np.zeros((128, GNN), np.float32)
    for b in range(3):
        pw1r[32 * b:32 * b + 5] = pw1
    d["pw1"] = pw1r
    w1s = np.zeros((N_SIN, GNN), np.float32)
    for k in range(N_SIN):
        c, i = rows[k]
        w1s[k] = msg_w1[c * 2 * HALF + i]
    d["w1s"] = w1s
    w1c = np.zeros((N_ANG, GNN), np.float32)
    for k in range(N_ANG):
        c, i = rows[k]
        w1c[k] = msg_w1[c * 2 * HALF + HALF + i]
    d["w1c"] = w1c
    w1hi = np.zeros((128, GNN), np.float32)
    for m in range(3):
        for c in range(3):
            for i in range(3):
                w1hi[32 * m + c * 3 + i] = msg_w1[c * 2 * HALF + i]
            w1hi[32 * m + 9 + c] = msg_w1[c * 2 * HALF + HALF + 0]
    d["w1hi"] = w1hi
    d["b1"] = np.stack([g("msg_b1")[0:128], g("msg_b1")[128:256]], 1)
    d["w2"] = g("msg_w2").reshape(2, 128, GNN).transpose(1, 0, 2).copy()
    d["projw"] = g("proj_w").reshape(2, 128, ENC).transpose(1, 0, 2).copy()
    projb = (g("msg_b2") @ g("proj_w") + g("proj_b")).astype(np.float32)
    d["projb"] = projb.reshape(3, 128).T.copy()

    for k, bp in enumerate(params["blocks"]):
        gb = lambda kk: np.asarray(bp[kk], np.float32)
        col3 = lambda v: np.asarray(v, np.float32).reshape(3, 128).T.copy()
        d[f"wq{k}"] = (gb("wq") / np.sqrt(HD)).reshape(3, 128, ENC).transpose(1, 0, 2).copy()
        d[f"bq{k}"] = col3(gb("bq") / np.sqrt(HD))
        d[f"wk{k}"] = gb("wk").reshape(3, 128, ENC).transpose(1, 0, 2).copy()
        d[f"bk{k}"] = col3(gb("bk"))
        wv = gb("wv")
        wvp = np.zeros((ENC, 390), np.float32)
        for h in range(HEADS):
            wvp[:, 65 * h:65 * h + 64] = wv[:, 64 * h:64 * (h + 1)]
        d[f"wvp{k}"] = wvp.reshape(3, 128, 390).transpose(1, 0, 2).copy()
        wo = gb("wo")
        woh = np.zeros((64, HEADS * ENC), np.float32)
        for h in range(HEADS):
            woh[:, h * ENC:(h + 1) * ENC] = wo[64 * h:64 * (h + 1), :]
        d[f"woh{k}"] = woh
        d[f"bo{k}"] = col3((gb("bv") @ wo + gb("bo")).astype(np.float32))
        d[f"ln1s{k}"] = col3(gb("ln1_s"))
        d[f"ln1b{k}"] = col3(gb("ln1_b"))
        d[f"ln2s{k}"] = col3(gb("ln2_s"))
        d[f"ln2b{k}"] = col3(gb("ln2_b"))
        d[f"w1m{k}"] = gb("mlp_w1").reshape(3, 128, 4 * ENC).transpose(1, 0, 2).copy()
        d[f"b1m{k}"] = gb("mlp_b1").reshape(12, 128).T.copy()
        d[f"w2m{k}"] = gb("mlp_w2").reshape(12, 128, ENC).transpose(1, 0, 2).copy()
        d[f"b2m{k}"] = col3(gb("mlp_b2"))
    return d


def _prep_sample(pts):
    freqs = _freqs()
    x5 = np.zeros((5, N), np.float32)
    x5[0:3] = pts.T
    x5[3] = (pts.astype(np.float64) ** 2).sum(1).astype(np.float32)
    x5[4] = 1.0
    ehi = np.zeros((128, 11264), np.float32)
    for m in range(3):
        chs = [ch for ch in range(64) if ch % 3 == m]
        seg = np.concatenate([pts[ch * 512:(ch + 1) * 512] for ch in chs], 0)
        w = seg.shape[0]
        for c in range(3):
            for i in range(3):
                ehi[32 * m + c * 3 + i, :w] = np.sin(seg[:, c] * freqs[i])
            ehi[32 * m + 9 + c, :w] = np.cos(seg[:, c] * freqs[0])
    return x5, ehi


def _prep_core(pts, sidx_half):
    sn = pts[sidx_half]
    sna1 = np.zeros((128, SOWN), np.float32)
    sna5 = np.zeros((128, SOWN), np.float32)
    for b in range(3):
        sna1[32 * b + 0:32 * b + 3] = 2.0 * sn.T
        sna1[32 * b + 3] = -1.0
        sna5[32 * b + 0:32 * b + 3] = 2.0 * sn.T
        sna5[32 * b + 3] = -1.0
    return sna1, sna5


def kernel(points, supernode_idxs, params, _dbg=False, _trace=False, _pmax=99, _nocc=False):
    points = np.asarray(points, np.float32)
    supernode_idxs = np.asarray(supernode_idxs)
    nc = _get_nc(_dbg, _pmax, _nocc)
    wd = _prep_weights(params)
    in_maps = []
    for c in range(8):
        s, h = c // 2, c % 2
        x5, ehi = _prep_sample(points[s])
        sna1, sna5 = _prep_core(points[s], supernode_idxs[s, h * SOWN:(h + 1) * SOWN])
        m = {"x5": x5, "sna1": sna1, "sna5": sna5, "ehi": ehi}
        m.update(wd)
        in_maps.append(m)
    res = run_bass_kernel_spmd(nc, in_maps, list(range(8)), trace=_trace,
                               trace_cores=list(range(8)) if _trace else None)
    outs = []
    for s in range(B):
        outs.append(np.concatenate(
            [res.results[2 * s]["out_x"], res.results[2 * s + 1]["out_x"]], 0))
    x = np.stack(outs)
    if _dbg:
        return (x, points), res
    return x, points
